# revision 13
# baseline (speedup 1.0000x reference)
"""MoE head (top-2 of 8 experts, GELU MLP, residual + LayerNorm) on 8 trn2
NeuronCores.

Strategy (expert-parallel):
  - Host: router (logits -> top-2 -> softmax), exactly as the reference
    computes it (fp32). Tokens are gathered per expert into capacity-padded
    buffers (capacity adapts to the actual max expert load, so nothing is
    ever dropped).
  - Device (8 cores, SPMD, core e owns expert e): y_e = (gelu(x_e @ W1_e
    + b1_e) @ W2_e + b2_e) * combine_weight.  All GEMMs run on the tensor
    engine in fp32 (fp32 accumulate in PSUM).  Activations are streamed
    token-major transposed (xT) so both GEMMs use natural weight layouts
    with zero on-device transposes.
  - Host: scatter-add the two expert contributions per token (pure
    unshard/combine), residual add + LayerNorm, reshape to [B, T, H].

Self-contained: hardcodes the nn_MoEHead problem shapes
(B=2, T=2048, H=1024, F=4096, E=8, top-2).
"""

import os
import sys
import types

import numpy as np


def _ensure_axon_ntff_hook():
    """bass_utils' axon trace path does `from antenv.axon_hooks import ...`;
    the container's antenv stub lacks that submodule, which would make any
    BASS_TRACE=1 run crash.  Recreate it, wiring the ctypes NTFF profiler
    hook from trn_agent_boot when available."""
    if "antenv.axon_hooks" in sys.modules:
        return
    mod = types.ModuleType("antenv.axon_hooks")
    hook = None
    try:
        from trn_agent_boot.trn_boot import _ntff_profile_via_ctypes

        so = "/opt/axon/libaxon_pjrt.so"
        if os.path.exists(so):
            hook = _ntff_profile_via_ctypes(so)
    except Exception:
        hook = None
    mod._hook = hook
    mod.get_axon_ntff_profile_hook = lambda: mod._hook

    def _set(h):
        mod._hook = h

    mod.set_axon_ntff_profile_hook = _set
    sys.modules["antenv.axon_hooks"] = mod
    try:
        import antenv

        antenv.axon_hooks = mod
    except Exception:
        pass


_ensure_axon_ntff_hook()

import concourse.bass as bass  # noqa: E402
import concourse.tile as tile  # noqa: E402
from concourse import bacc, mybir  # noqa: E402
from concourse.bass_utils import run_bass_kernel_spmd  # noqa: E402

P = 128
H = 1024
F = 4096
E = 8
TOP_K = 2
LN_EPS = 1e-5
KO = H // P  # 8  k-tiles for GEMM1 (contraction over H)
FO = F // P  # 32 f-tiles
HO = H // P  # 8  h-tiles of the output
F_BLK = 4  # f-tiles per F block (512 wide)
N_FBLK = FO // F_BLK  # 8
TOK_B = 512  # max token block (psum free-dim limit for fp32)

# "f32"  : exact fp32 matmuls (4 cycles/row on the PE)
# "f32r" : fp32 data, relaxed-precision PE mode (1 cycle/row, ~1e-3 rel err)
MM_DT = os.environ.get("MOE_MM_DT", "f32r")

_kernel_cache: dict = {}


def _tok_blocks(C):
    """Split C tokens (a multiple of 16) into near-equal 16-aligned blocks
    of <=512 (and >=256 when C allows) — wide moving operands keep the PE
    at full rate, and 16-element alignment keeps the ISA happy."""
    assert C % 16 == 0
    nb = max(1, -(-C // TOK_B))
    n16 = C // 16
    sizes = [16 * (n16 // nb + (1 if i < n16 % nb else 0)) for i in range(nb)]
    blocks = []
    off = 0
    for sz in sizes:
        blocks.append((off, sz))
        off += sz
    return blocks


def _build_moe_kernel(C, mm_dt):
    """One expert's FFN over C capacity-padded tokens.

    in : xT [H, C], w1 [H, F], b1v [F], w2 [F, H], b2v [H], wgt [C]
    out: yT [H, C] = ((gelu(xT.T @ w1 + b1) @ w2) + b2).T * wgt
    """
    f32 = mybir.dt.float32
    # In f32r mode the matmul operands (activations + weights) are typed
    # float32r end-to-end: the BIR verifier requires every producer of an
    # FP32r-matmul operand to round to FP32r.
    mdt = mybir.dt.float32r if mm_dt == "f32r" else f32
    nc = bacc.Bacc(None, target_bir_lowering=False, debug=False)

    xT = nc.dram_tensor("xT", [H, C], mdt, kind="ExternalInput")
    w1 = nc.dram_tensor("w1", [H, F], mdt, kind="ExternalInput")
    b1v = nc.dram_tensor("b1v", [F], f32, kind="ExternalInput")
    w2 = nc.dram_tensor("w2", [F, H], mdt, kind="ExternalInput")
    b2v = nc.dram_tensor("b2v", [H], f32, kind="ExternalInput")
    wgt = nc.dram_tensor("wgt", [C], f32, kind="ExternalInput")
    yT = nc.dram_tensor("yT", [H, C], f32, kind="ExternalOutput")

    xT_r = xT.rearrange("(ko p) c -> p ko c", p=P)  # [128, 8, C]
    w1_r = w1.rearrange("(ko p) f -> p ko f", p=P)  # [128, 8, F]
    w2_r = w2.rearrange("(fo p) h -> p fo h", p=P)  # [128, 32, H]
    b1_r = b1v.rearrange("(fo p) -> p fo", p=P)  # [128, 32]
    b2_r = b2v.rearrange("(ho p) -> p ho", p=P)  # [128, 8]
    yT_r = yT.rearrange("(ho p) c -> p ho c", p=P)  # [128, 8, C]

    blocks = _tok_blocks(C)

    with tile.TileContext(nc) as tc:
        with (
            tc.tile_pool(name="singles", bufs=1) as singles,
            tc.tile_pool(name="w1p", bufs=2) as w1p,
            tc.tile_pool(name="w2p", bufs=2) as w2p,
            tc.tile_pool(name="hp", bufs=1) as hp,
            tc.tile_pool(name="ps1", bufs=3, space="PSUM") as ps1,
            tc.tile_pool(name="ps2", bufs=4, space="PSUM") as ps2,
        ):
            def dma_w1(fb, nsplit=1):
                t = w1p.tile([P, KO, F_BLK * P], mdt, name="w1_sb")
                ks = KO // nsplit
                for s in range(nsplit):
                    nc.sync.dma_start(
                        t[:, s * ks : (s + 1) * ks, :],
                        w1_r[
                            :,
                            s * ks : (s + 1) * ks,
                            fb * F_BLK * P : (fb + 1) * F_BLK * P,
                        ],
                    )
                return t

            def dma_w2(fb, nsplit=1):
                t = w2p.tile([P, F_BLK, H], mdt, name="w2_sb")
                fs = F_BLK // nsplit
                for s in range(nsplit):
                    nc.sync.dma_start(
                        t[:, s * fs : (s + 1) * fs, :],
                        w2_r[:, fb * F_BLK + s * fs : fb * F_BLK + (s + 1) * fs, :],
                    )
                return t

            # Emission order == DMA priority: first f-block's W1 and the
            # first token block go first so the PE starts ~6us in, the rest
            # of the activations stream behind, W2 before GEMM2 needs it.
            w1_first = dma_w1(0, nsplit=4)
            xT_sbs = []
            for bi, (off, sz) in enumerate(blocks):
                t = singles.tile([P, KO, sz], mdt, tag=f"xT{bi}", name=f"xT{bi}")
                for s in range(2):
                    ks = KO // 2
                    nc.sync.dma_start(
                        t[:, s * ks : (s + 1) * ks, :],
                        xT_r[:, s * ks : (s + 1) * ks, off : off + sz],
                    )
                xT_sbs.append(t)
                if bi == 0:
                    b1_sb = singles.tile([P, FO], f32)
                    nc.sync.dma_start(b1_sb[:], b1_r[:])
            w2_first = dma_w2(0, nsplit=2)
            b2_sb = singles.tile([P, HO], f32)
            nc.sync.dma_start(b2_sb[:], b2_r[:])
            # combine weights broadcast across partitions: [128, C]
            # (on the gpsimd SWDGE queue, needed only by the last f-block)
            wgt_sb = singles.tile([P, C], f32)
            wgt_ap = wgt[:]
            wgt_bc = bass.AP(
                tensor=wgt_ap.tensor,
                offset=wgt_ap.offset,
                ap=[[0, P], *wgt_ap.ap],
            )
            nc.gpsimd.dma_start(out=wgt_sb[:], in_=wgt_bc)

            # output accumulator, one tile per (h tile, token block)
            yacc = [
                [
                    singles.tile([P, sz], f32, tag=f"y{ho}_{bi}", name=f"y{ho}_{bi}")
                    for bi, (off, sz) in enumerate(blocks)
                ]
                for ho in range(HO)
            ]

            for fb in range(N_FBLK):
                w1_sb = w1_first if fb == 0 else dma_w1(fb)
                w2_sb = w2_first if fb == 0 else dma_w2(fb)
                # hT split per token block for precise gelu->GEMM2 deps
                hTs = [
                    hp.tile([P, F_BLK, sz], mdt, tag=f"hT{bi}", name=f"hT{bi}")
                    for bi, (off, sz) in enumerate(blocks)
                ]

                # GEMM1: hT[f_tile, tok] = gelu(sum_k w1[k, f_tile].T @ xT[k, tok] + b1)
                # (token-block-major: the first matmuls only need xT block 0,
                # so the PE starts while the other blocks are still in flight)
                for bi, (off, sz) in enumerate(blocks):
                    for ft in range(F_BLK):
                        psum = ps1.tile([P, TOK_B], f32)
                        for k in range(KO):
                            nc.tensor.matmul(
                                psum[:, :sz],
                                w1_sb[:, k, ft * P : (ft + 1) * P],
                                xT_sbs[bi][:, k, :],
                                start=(k == 0),
                                stop=(k == KO - 1),
                            )
                        nc.scalar.activation(
                            hTs[bi][:, ft, :],
                            psum[:, :sz],
                            mybir.ActivationFunctionType.Gelu,
                            bias=b1_sb[:, fb * F_BLK + ft : fb * F_BLK + ft + 1],
                        )

                # GEMM2 partial: y[h_tile, tok] += sum_ft w2[ft, h_tile].T @ hT[ft, tok]
                for ho in range(HO):
                    for bi, (off, sz) in enumerate(blocks):
                        psum2 = ps2.tile([P, TOK_B], f32)
                        for ft in range(F_BLK):
                            nc.tensor.matmul(
                                psum2[:, :sz],
                                w2_sb[:, ft, ho * P : (ho + 1) * P],
                                hTs[bi][:, ft, :],
                                start=(ft == 0),
                                stop=(ft == F_BLK - 1),
                            )
                        ya = yacc[ho][bi]
                        if fb == 0:
                            # fold the b2 bias into the first accumulate
                            nc.vector.tensor_scalar_add(
                                ya[:], psum2[:, :sz], b2_sb[:, ho : ho + 1]
                            )
                        else:
                            nc.vector.tensor_add(ya[:], ya[:], psum2[:, :sz])
                        if fb == N_FBLK - 1:
                            # epilogue per chunk: combine-weight scale + store
                            nc.vector.tensor_mul(
                                ya[:], ya[:], wgt_sb[:, off : off + sz]
                            )
                            nc.sync.dma_start(yT_r[:, ho, off : off + sz], ya[:])

    nc.compile()
    return nc


def _get_kernel(C, mm_dt):
    key = (C, mm_dt)
    if key not in _kernel_cache:
        _kernel_cache[key] = _build_moe_kernel(C, mm_dt)
    return _kernel_cache[key]


def _route(x, router_w, router_b):
    """Replicates the reference router bit-for-bit up to fp32 matmul
    rounding: logits -> top-2 (ties to lower index) -> softmax."""
    logits = x @ router_w.T + router_b  # [N, E] fp32
    order = np.argsort(-logits, axis=-1, kind="stable")
    idx = order[:, :TOP_K]  # [N, 2]
    vals = np.take_along_axis(logits, idx, axis=-1)
    vmax = vals.max(axis=-1, keepdims=True)
    ex = np.exp(vals - vmax)
    w = ex / ex.sum(axis=-1, keepdims=True)
    return idx, w.astype(np.float32)


def kernel(
    hidden_states,
    router_w,
    router_b,
    W1,
    b1,
    W2,
    b2,
    ln_gamma,
    ln_beta,
):
    hidden_states = np.asarray(hidden_states, np.float32)
    router_w = np.asarray(router_w, np.float32)
    router_b = np.asarray(router_b, np.float32)
    W1 = np.asarray(W1, np.float32)
    b1 = np.asarray(b1, np.float32)
    W2 = np.asarray(W2, np.float32)
    b2 = np.asarray(b2, np.float32)
    ln_gamma = np.asarray(ln_gamma, np.float32)
    ln_beta = np.asarray(ln_beta, np.float32)

    B, T, Hdim = hidden_states.shape
    N = B * T
    x = np.ascontiguousarray(hidden_states.reshape(N, Hdim))

    idx, topw = _route(x, router_w, router_b)

    tok_ids = np.arange(N)
    toks_per_e = []
    wts_per_e = []
    for e in range(E):
        sel0 = idx[:, 0] == e
        sel1 = idx[:, 1] == e
        toks = np.concatenate([tok_ids[sel0], tok_ids[sel1]])
        ws = np.concatenate([topw[sel0, 0], topw[sel1, 1]])
        toks_per_e.append(toks)
        wts_per_e.append(ws)

    max_cnt = max(len(t) for t in toks_per_e)
    # capacity: multiple of 16 keeps DMA rows 64B-aligned; >=256 keeps the
    # PE at full rate in f32r mode
    C = max(((max_cnt + 15) // 16) * 16, 256)

    nc = _get_kernel(C, MM_DT)

    in_maps = []
    for e in range(E):
        toks = toks_per_e[e]
        n = len(toks)
        X = np.zeros((C, Hdim), dtype=np.float32)
        X[:n] = x[toks]
        wv = np.zeros((C,), dtype=np.float32)
        wv[:n] = wts_per_e[e]
        in_maps.append(
            {
                "xT": np.ascontiguousarray(X.T),
                "w1": np.ascontiguousarray(np.asarray(W1[e], np.float32)),
                "b1v": np.ascontiguousarray(np.asarray(b1[e], np.float32)),
                "w2": np.ascontiguousarray(np.asarray(W2[e], np.float32)),
                "b2v": np.ascontiguousarray(np.asarray(b2[e], np.float32)),
                "wgt": wv,
            }
        )

    res = run_bass_kernel_spmd(nc, in_maps, core_ids=list(range(E)))

    out = np.zeros((N, Hdim), dtype=np.float64)
    for e in range(E):
        toks = toks_per_e[e]
        n = len(toks)
        yT = res.results[e]["yT"]  # [H, C]
        out[toks] += yT.T[:n].astype(np.float64)

    # residual + LayerNorm (float64 internally; reference is fp32)
    out += x.astype(np.float64)
    mu = out.mean(axis=-1, keepdims=True)
    var = out.var(axis=-1, keepdims=True)
    out = (out - mu) / np.sqrt(var + LN_EPS)
    out = out * np.asarray(ln_gamma, np.float64) + np.asarray(ln_beta, np.float64)

    return out.astype(np.float32).reshape(B, T, Hdim)


# revision 14
# speedup vs baseline: 1.1773x; 1.1773x over previous
"""MoE head (top-2 of 8 experts, GELU MLP, residual + LayerNorm) on 8 trn2
NeuronCores.

Strategy (expert-parallel):
  - Host: router (logits -> top-2 -> softmax), exactly as the reference
    computes it (fp32). Tokens are gathered per expert into capacity-padded
    buffers (capacity adapts to the actual max expert load, so nothing is
    ever dropped).
  - Device (8 cores, SPMD, core e owns expert e): y_e = (gelu(x_e @ W1_e
    + b1_e) @ W2_e + b2_e) * combine_weight.  All GEMMs run on the tensor
    engine in fp32 (fp32 accumulate in PSUM).  Activations are streamed
    token-major transposed (xT) so both GEMMs use natural weight layouts
    with zero on-device transposes.
  - Host: scatter-add the two expert contributions per token (pure
    unshard/combine), residual add + LayerNorm, reshape to [B, T, H].

Self-contained: hardcodes the nn_MoEHead problem shapes
(B=2, T=2048, H=1024, F=4096, E=8, top-2).
"""

import os
import sys
import types

import numpy as np


def _ensure_axon_ntff_hook():
    """bass_utils' axon trace path does `from antenv.axon_hooks import ...`;
    the container's antenv stub lacks that submodule, which would make any
    BASS_TRACE=1 run crash.  Recreate it, wiring the ctypes NTFF profiler
    hook from trn_agent_boot when available."""
    if "antenv.axon_hooks" in sys.modules:
        return
    mod = types.ModuleType("antenv.axon_hooks")
    hook = None
    try:
        from trn_agent_boot.trn_boot import _ntff_profile_via_ctypes

        so = "/opt/axon/libaxon_pjrt.so"
        if os.path.exists(so):
            hook = _ntff_profile_via_ctypes(so)
    except Exception:
        hook = None
    mod._hook = hook
    mod.get_axon_ntff_profile_hook = lambda: mod._hook

    def _set(h):
        mod._hook = h

    mod.set_axon_ntff_profile_hook = _set
    sys.modules["antenv.axon_hooks"] = mod
    try:
        import antenv

        antenv.axon_hooks = mod
    except Exception:
        pass


_ensure_axon_ntff_hook()

import concourse.bass as bass  # noqa: E402
import concourse.tile as tile  # noqa: E402
from concourse import bacc, mybir  # noqa: E402
from concourse.bass_utils import run_bass_kernel_spmd  # noqa: E402

P = 128
H = 1024
F = 4096
E = 8
TOP_K = 2
LN_EPS = 1e-5
KO = H // P  # 8  k-tiles for GEMM1 (contraction over H)
FO = F // P  # 32 f-tiles
HO = H // P  # 8  h-tiles of the output
F_BLK = 4  # f-tiles per F block (512 wide)
N_FBLK = FO // F_BLK  # 8
TOK_B = 512  # max token block (psum free-dim limit for fp32)

# "f32"  : exact fp32 matmuls (4 cycles/row on the PE)
# "f32r" : fp32 data, relaxed-precision PE mode (1 cycle/row, ~1e-3 rel err)
MM_DT = os.environ.get("MOE_MM_DT", "f32r")

_kernel_cache: dict = {}


def _tok_blocks(C):
    """Split C tokens (a multiple of 16) into near-equal 16-aligned blocks
    of <=512 (and >=256 when C allows) — wide moving operands keep the PE
    at full rate, and 16-element alignment keeps the ISA happy."""
    assert C % 16 == 0
    nb = max(1, -(-C // TOK_B))
    n16 = C // 16
    sizes = [16 * (n16 // nb + (1 if i < n16 % nb else 0)) for i in range(nb)]
    blocks = []
    off = 0
    for sz in sizes:
        blocks.append((off, sz))
        off += sz
    return blocks


def _build_moe_kernel(C, mm_dt):
    """One expert's FFN over C capacity-padded tokens.

    in : xT [H, C], w1 [H, F], b1v [F], w2 [F, H], b2v [H], wgt [C]
    out: yT [H, C] = ((gelu(xT.T @ w1 + b1) @ w2) + b2).T * wgt
    """
    f32 = mybir.dt.float32
    # In f32r mode the matmul operands (activations + weights) are typed
    # float32r end-to-end: the BIR verifier requires every producer of an
    # FP32r-matmul operand to round to FP32r.
    mdt = mybir.dt.float32r if mm_dt == "f32r" else f32
    nc = bacc.Bacc(None, target_bir_lowering=False, debug=False)

    xT = nc.dram_tensor("xT", [H, C], mdt, kind="ExternalInput")
    w1 = nc.dram_tensor("w1", [H, F], mdt, kind="ExternalInput")
    b1v = nc.dram_tensor("b1v", [F], f32, kind="ExternalInput")
    w2 = nc.dram_tensor("w2", [F, H], mdt, kind="ExternalInput")
    b2v = nc.dram_tensor("b2v", [H], f32, kind="ExternalInput")
    wgt = nc.dram_tensor("wgt", [C], f32, kind="ExternalInput")
    yT = nc.dram_tensor("yT", [H, C], f32, kind="ExternalOutput")

    xT_r = xT.rearrange("(ko p) c -> p ko c", p=P)  # [128, 8, C]
    w1_r = w1.rearrange("(ko p) f -> p ko f", p=P)  # [128, 8, F]
    w2_r = w2.rearrange("(fo p) h -> p fo h", p=P)  # [128, 32, H]
    b1_r = b1v.rearrange("(fo p) -> p fo", p=P)  # [128, 32]
    b2_r = b2v.rearrange("(ho p) -> p ho", p=P)  # [128, 8]
    yT_r = yT.rearrange("(ho p) c -> p ho c", p=P)  # [128, 8, C]

    blocks = _tok_blocks(C)

    with tile.TileContext(nc) as tc:
        with (
            tc.tile_pool(name="singles", bufs=1) as singles,
            tc.tile_pool(name="w1p", bufs=2) as w1p,
            tc.tile_pool(name="w2p", bufs=2) as w2p,
            tc.tile_pool(name="hp", bufs=1) as hp,
            tc.tile_pool(name="ps1", bufs=3, space="PSUM") as ps1,
            tc.tile_pool(name="ps2", bufs=4, space="PSUM") as ps2,
        ):
            def dma_w1(fb):
                t = w1p.tile([P, KO, F_BLK * P], mdt, name="w1_sb")
                nc.sync.dma_start(
                    t[:], w1_r[:, :, fb * F_BLK * P : (fb + 1) * F_BLK * P]
                )
                return t

            def dma_w2(fb):
                t = w2p.tile([P, F_BLK, H], mdt, name="w2_sb")
                nc.sync.dma_start(t[:], w2_r[:, fb * F_BLK : (fb + 1) * F_BLK, :])
                return t

            # Emission order == DMA priority: first f-block's W1 and the
            # first token block go first so the PE starts ~6us in, the rest
            # of the activations stream behind, W2 before GEMM2 needs it.
            w1_first = dma_w1(0)
            xT_sbs = []
            for bi, (off, sz) in enumerate(blocks):
                t = singles.tile([P, KO, sz], mdt, tag=f"xT{bi}", name=f"xT{bi}")
                nc.sync.dma_start(t[:], xT_r[:, :, off : off + sz])
                xT_sbs.append(t)
                if bi == 0:
                    b1_sb = singles.tile([P, FO], f32)
                    nc.sync.dma_start(b1_sb[:], b1_r[:])
            w2_first = dma_w2(0)
            b2_sb = singles.tile([P, HO], f32)
            nc.sync.dma_start(b2_sb[:], b2_r[:])
            # combine weights broadcast across partitions: [128, C]
            # (on the gpsimd SWDGE queue, needed only by the last f-block)
            wgt_sb = singles.tile([P, C], f32)
            wgt_ap = wgt[:]
            wgt_bc = bass.AP(
                tensor=wgt_ap.tensor,
                offset=wgt_ap.offset,
                ap=[[0, P], *wgt_ap.ap],
            )
            nc.gpsimd.dma_start(out=wgt_sb[:], in_=wgt_bc)

            # output accumulator, one tile per (h tile, token block)
            yacc = [
                [
                    singles.tile([P, sz], f32, tag=f"y{ho}_{bi}", name=f"y{ho}_{bi}")
                    for bi, (off, sz) in enumerate(blocks)
                ]
                for ho in range(HO)
            ]

            for fb in range(N_FBLK):
                w1_sb = w1_first if fb == 0 else dma_w1(fb)
                w2_sb = w2_first if fb == 0 else dma_w2(fb)
                # hT split per token block for precise gelu->GEMM2 deps
                hTs = [
                    hp.tile([P, F_BLK, sz], mdt, tag=f"hT{bi}", name=f"hT{bi}")
                    for bi, (off, sz) in enumerate(blocks)
                ]

                # GEMM1: hT[f_tile, tok] = gelu(sum_k w1[k, f_tile].T @ xT[k, tok] + b1)
                # (token-block-major: the first matmuls only need xT block 0,
                # so the PE starts while the other blocks are still in flight)
                for bi, (off, sz) in enumerate(blocks):
                    for ft in range(F_BLK):
                        psum = ps1.tile([P, TOK_B], f32)
                        for k in range(KO):
                            nc.tensor.matmul(
                                psum[:, :sz],
                                w1_sb[:, k, ft * P : (ft + 1) * P],
                                xT_sbs[bi][:, k, :],
                                start=(k == 0),
                                stop=(k == KO - 1),
                            )
                        nc.scalar.activation(
                            hTs[bi][:, ft, :],
                            psum[:, :sz],
                            mybir.ActivationFunctionType.Gelu,
                            bias=b1_sb[:, fb * F_BLK + ft : fb * F_BLK + ft + 1],
                        )

                # GEMM2 partial: y[h_tile, tok] += sum_ft w2[ft, h_tile].T @ hT[ft, tok]
                for ho in range(HO):
                    for bi, (off, sz) in enumerate(blocks):
                        psum2 = ps2.tile([P, TOK_B], f32)
                        for ft in range(F_BLK):
                            nc.tensor.matmul(
                                psum2[:, :sz],
                                w2_sb[:, ft, ho * P : (ho + 1) * P],
                                hTs[bi][:, ft, :],
                                start=(ft == 0),
                                stop=(ft == F_BLK - 1),
                            )
                        ya = yacc[ho][bi]
                        if fb == 0:
                            # fold the b2 bias into the first accumulate
                            nc.vector.tensor_scalar_add(
                                ya[:], psum2[:, :sz], b2_sb[:, ho : ho + 1]
                            )
                        else:
                            nc.vector.tensor_add(ya[:], ya[:], psum2[:, :sz])
                        if fb == N_FBLK - 1:
                            # epilogue per chunk: combine-weight scale + store
                            nc.vector.tensor_mul(
                                ya[:], ya[:], wgt_sb[:, off : off + sz]
                            )
                            nc.sync.dma_start(yT_r[:, ho, off : off + sz], ya[:])

    nc.compile()
    return nc


def _get_kernel(C, mm_dt):
    key = (C, mm_dt)
    if key not in _kernel_cache:
        _kernel_cache[key] = _build_moe_kernel(C, mm_dt)
    return _kernel_cache[key]


def _route(x, router_w, router_b):
    """Replicates the reference router bit-for-bit up to fp32 matmul
    rounding: logits -> top-2 (ties to lower index) -> softmax."""
    logits = x @ router_w.T + router_b  # [N, E] fp32
    order = np.argsort(-logits, axis=-1, kind="stable")
    idx = order[:, :TOP_K]  # [N, 2]
    vals = np.take_along_axis(logits, idx, axis=-1)
    vmax = vals.max(axis=-1, keepdims=True)
    ex = np.exp(vals - vmax)
    w = ex / ex.sum(axis=-1, keepdims=True)
    return idx, w.astype(np.float32)


def kernel(
    hidden_states,
    router_w,
    router_b,
    W1,
    b1,
    W2,
    b2,
    ln_gamma,
    ln_beta,
):
    hidden_states = np.asarray(hidden_states, np.float32)
    router_w = np.asarray(router_w, np.float32)
    router_b = np.asarray(router_b, np.float32)
    W1 = np.asarray(W1, np.float32)
    b1 = np.asarray(b1, np.float32)
    W2 = np.asarray(W2, np.float32)
    b2 = np.asarray(b2, np.float32)
    ln_gamma = np.asarray(ln_gamma, np.float32)
    ln_beta = np.asarray(ln_beta, np.float32)

    B, T, Hdim = hidden_states.shape
    N = B * T
    x = np.ascontiguousarray(hidden_states.reshape(N, Hdim))

    idx, topw = _route(x, router_w, router_b)

    tok_ids = np.arange(N)
    toks_per_e = []
    wts_per_e = []
    for e in range(E):
        sel0 = idx[:, 0] == e
        sel1 = idx[:, 1] == e
        toks = np.concatenate([tok_ids[sel0], tok_ids[sel1]])
        ws = np.concatenate([topw[sel0, 0], topw[sel1, 1]])
        toks_per_e.append(toks)
        wts_per_e.append(ws)

    max_cnt = max(len(t) for t in toks_per_e)
    # capacity: multiple of 16 keeps DMA rows 64B-aligned; >=256 keeps the
    # PE at full rate in f32r mode
    C = max(((max_cnt + 15) // 16) * 16, 256)

    nc = _get_kernel(C, MM_DT)

    in_maps = []
    for e in range(E):
        toks = toks_per_e[e]
        n = len(toks)
        X = np.zeros((C, Hdim), dtype=np.float32)
        X[:n] = x[toks]
        wv = np.zeros((C,), dtype=np.float32)
        wv[:n] = wts_per_e[e]
        in_maps.append(
            {
                "xT": np.ascontiguousarray(X.T),
                "w1": np.ascontiguousarray(np.asarray(W1[e], np.float32)),
                "b1v": np.ascontiguousarray(np.asarray(b1[e], np.float32)),
                "w2": np.ascontiguousarray(np.asarray(W2[e], np.float32)),
                "b2v": np.ascontiguousarray(np.asarray(b2[e], np.float32)),
                "wgt": wv,
            }
        )

    res = run_bass_kernel_spmd(nc, in_maps, core_ids=list(range(E)))

    out = np.zeros((N, Hdim), dtype=np.float64)
    for e in range(E):
        toks = toks_per_e[e]
        n = len(toks)
        yT = res.results[e]["yT"]  # [H, C]
        out[toks] += yT.T[:n].astype(np.float64)

    # residual + LayerNorm (float64 internally; reference is fp32)
    out += x.astype(np.float64)
    mu = out.mean(axis=-1, keepdims=True)
    var = out.var(axis=-1, keepdims=True)
    out = (out - mu) / np.sqrt(var + LN_EPS)
    out = out * np.asarray(ln_gamma, np.float64) + np.asarray(ln_beta, np.float64)

    return out.astype(np.float32).reshape(B, T, Hdim)
